# revision 53
# baseline (speedup 1.0000x reference)
"""MultiHeadAttention TRN2 kernel: B=2, S=2048, D=1024, H=16, DK=64, 8 cores.

Sharding: core c handles batch b=c//4 and heads hg=(c%4)*4 .. +3 (data + head
parallel). Projections column-split by head; out-proj row-split; the
all-reduce after out-proj is done on host (sum of 4 partials per batch).

v2 design (cost-model driven):
- q/k/v projections run as fp8e4m3 hi+lo DoubleRow matmuls (2 contraction
  chunks per instr at 0.5 cyc/row): 3 product terms (xh*wh, xh*wl, xl*wh)
  give ~0.15% proj error while cutting proj PE time 25% and halving input
  DMA bytes. Weights are host-prescaled by 16 (fp8 subnormal avoidance);
  the 1/16 scales fold into the exp scale (q,k) and the ones column (v).
- Attention scores stay fp32r; exp output (probs) and v are bf16 (matmul
  still 1 cyc/row, ~0.3% extra error well inside the budget, and half the
  SBUF -- funding 13-deep expT and 8-deep input-block buffering). ctx is
  accumulated over chunks with a ones column (=16) appended to v for the
  softmax denominator.
- Streaming schedule: the first exp fires ~14us in (bq+wk+K0+wq+Q0 is
  the serialized-DMA critical prefix); projection/out-proj work is
  injected into per-chunk slots of the attention sweeps; scores are
  issued one chunk ahead of ctx so ACT rarely waits on PE.
- PSUM: s_ps0/1 (2 banks each, scores double-buffer), cu0/1 (ctx accum;
  fast norm drain via Pool den-broadcast frees them within ~1.5us), and
  fp0/1 (projections + out-proj, 2-deep so matmul overlaps evac) = 8
  banks. Emission order doubles as scheduler priority and as the psum
  tag-rotation order, so projection items are emitted in DMA-arrival
  order and the exp/scores/ctx mainline carries a large priority boost.
"""

from contextlib import ExitStack

import numpy as np

B, S, D, H, DK = 2, 2048, 1024, 16, 64
NCORES = 8
HPC = H // (NCORES // B)      # heads per core = 4
R = HPC * DK                  # local feats = 256
NKC = S // 128                # k-chunks per sweep = 16
NQB = 4                       # 512-wide q blocks
VW = 65                       # v chunk width (64 + ones col)
NSW = 8                       # sweeps = NQB * 2 head-pairs
EXPSCALE = 0.125 / 256.0      # qT,kT hold 16x values

_CACHE = {}
_LAST_IN_MAPS = None


def _build():
    import concourse.mybir as mybir
    import concourse.tile as tile
    from concourse import bacc

    f32 = mybir.dt.float32
    f32r = mybir.dt.float32r
    f8 = mybir.dt.float8e4
    DR = mybir.MatmulPerfMode.DoubleRow
    Exp = mybir.ActivationFunctionType.Exp
    Add = mybir.AluOpType.add

    nc = bacc.Bacc(
        "TRN2", target_bir_lowering=False, debug=False,
        enable_asserts=True, num_devices=NCORES,
    )

    # hi/lo fp8 planes interleaved at block granularity (3-dim DMA APs):
    # x2 [D, 4blk*(2plane*512)], w2 [D, 2plane*256] -- one DMA per block
    x_d = {}
    for nm in ("Q2", "K2", "V2"):
        x_d[nm] = nc.dram_tensor(nm, [D, 2 * S], f8, kind="ExternalInput").ap()
    w_d = {}
    for nm in ("wq2", "wk2", "wv2"):
        w_d[nm] = nc.dram_tensor(nm, [D, 2 * R], f8, kind="ExternalInput").ap()
    woT_d = nc.dram_tensor("woT", [R, D], f32r, kind="ExternalInput").ap()
    bq_d = nc.dram_tensor("bq16", [R, 1], f32, kind="ExternalInput").ap()
    out_d = nc.dram_tensor("OUT", [S, D], mybir.dt.bfloat16, kind="ExternalOutput").ap()

    with tile.TileContext(nc) as tc, ExitStack() as ctx:
        sb = ctx.enter_context(tc.tile_pool(name="sb", bufs=1))
        xin = ctx.enter_context(tc.tile_pool(name="xin", bufs=8))
        expp = ctx.enter_context(tc.tile_pool(name="expp", bufs=13))
        normp = ctx.enter_context(tc.tile_pool(name="normp", bufs=3))
        osb = ctx.enter_context(tc.tile_pool(name="osb", bufs=3))
        psum = ctx.enter_context(tc.tile_pool(name="psum", bufs=1, space="PSUM"))

        cnt = {"s": 0, "c": 0, "f": 0}

        def s_tile():
            i = cnt["s"]; cnt["s"] += 1
            return psum.tile([128, 1024], f32, name=f"s_ps{i % 2}", tag=f"s_ps{i % 2}")

        def fk_tile(w=512):
            i = cnt["f"]; cnt["f"] += 1
            return psum.tile([128, w], f32, name=f"fp{i % 2}", tag=f"fp{i % 2}")

        fv_tile = fk_tile

        def cu_tile():
            i = cnt["c"]; cnt["c"] += 1
            return psum.tile([128, 512], f32, name=f"cu{i % 2}", tag=f"cu{i % 2}")

        # ---- persistent SBUF ----
        w_sb = {nm: sb.tile([128, 8 * 2 * R], f8, name=f"{nm}_sb") for nm in w_d}
        bq_sb = sb.tile([128, 2], f32)
        wo_sb = [sb.tile([128, D], f32r, name=f"wo_sb{cn}") for cn in range(2)]
        qT_sb = [sb.tile([128, S], f32r, name=f"qT_sb{hp}") for hp in range(2)]
        kT_sb = [sb.tile([128, S], f32r, name=f"kT_sb{hp}") for hp in range(2)]
        v_all = sb.tile([128, HPC * NKC * VW], mybir.dt.bfloat16)
        ctxT_sb = [sb.tile([128, S], f32r, name=f"ctxT_sb{cn}") for cn in range(2)]

        onecol = sb.tile([128, 1], f32)
        nc.vector.memset(onecol[:], 16.0)   # absorbs the 1/16 of fp8 v scale
        vv = v_all.rearrange("p (n c) -> p n c", c=VW)[:, :, 64:65].rearrange(
            "p n c -> p (n c)")
        nc.vector.tensor_copy(vv, onecol[:].broadcast_to((128, HPC * NKC)))

        # ---- DMA: single queue, strict deadline order (the DMA engines
        # device serializes transfers globally at ~2.9us per merged block) ----
        def w_load(eng, nm):
            eng.dma_start(
                w_sb[nm].rearrange("p (d r2) -> p d r2", d=8),
                w_d[nm].rearrange("(d p) r2 -> p d r2", p=128))

        # x block tiles: [128, 8 dchunks, 2*512 (H cols | L cols)] fp8
        xtiles = {}

        def x_load(eng, nm, blk):
            t = xin.tile([128, 8, 1024], f8, name="xin", tag="xin")
            eng.dma_start(
                t[:],
                x_d[nm].rearrange("(d p) c -> p d c", p=128)[
                    :, :, 1024 * blk:1024 * (blk + 1)])
            xtiles[(nm, blk)] = t

        for hp in range(2):
            nc.sync.dma_start(bq_sb[:, hp:hp + 1], bq_d[128 * hp:128 * (hp + 1), :])
        w_load(nc.sync, "wk2")
        x_load(nc.sync, "K2", 0)
        w_load(nc.sync, "wq2")
        x_load(nc.sync, "Q2", 0)
        w_load(nc.sync, "wv2")
        x_load(nc.sync, "V2", 0)
        for blk in range(1, 4):
            x_load(nc.sync, "K2", blk)
            x_load(nc.sync, "V2", blk)
        x_load(nc.sync, "Q2", 1)
        for cn in range(2):
            nc.sync.dma_start(wo_sb[cn][:], woT_d[128 * cn:128 * (cn + 1), :])
        for blk in range(2, 4):
            x_load(nc.sync, "Q2", blk)

        # ---- fp8 hi/lo DoubleRow projection: 12 matmuls per psum tile ----
        TERMS = (("H", "H"), ("H", "L"), ("L", "H"))

        PL = {"H": 0, "L": 1}

        def proj_mms(p_ps, xnm, blk, wnm, hp, out_sl=slice(0, 512)):
            n = 0
            xt = xtiles[(xnm, blk)]
            wt = w_sb[wnm].rearrange("p (d r2) -> p d r2", d=8)
            for xa, wb in TERMS:
                wo_ = 256 * PL[wb] + 128 * hp
                xo = 512 * PL[xa]
                for j in range(4):
                    nc.tensor.matmul(
                        p_ps[:, out_sl],
                        wt[:, 2 * j:2 * j + 2, wo_:wo_ + 128],
                        xt[:, 2 * j:2 * j + 2, xo:xo + 512],
                        start=(n == 0), stop=(n == 11), perf_mode=DR)
                    n += 1

        def qk_item(xnm, wnm, dst_sb, blk, hp, bias):
            def go():
              from contextlib import nullcontext
              boost = (tc.high_priority(offset=500000) if xnm == "K2"
                       else nullcontext())
              with boost:
                p_ps = fk_tile()
                proj_mms(p_ps, xnm, blk, wnm, hp)
                dst = dst_sb[hp][:, 512 * blk:512 * (blk + 1)]
                if bias:
                    nc.vector.tensor_scalar(
                        dst, p_ps[:], bq_sb[:, hp:hp + 1], None, op0=Add)
                else:
                    nc.vector.tensor_copy(dst, p_ps[:])
            return go

        def v_item(c):
            def go():
              with tc.high_priority(offset=500000):
                blk, sub = c // 4, c % 4
                v_ps = fv_tile(256)
                n = 0
                xt = xtiles[("V2", blk)]
                wt = w_sb["wv2"].rearrange("p (d r2) -> p d r2", d=8)
                for xa, wb in TERMS:
                    xo = 512 * PL[xa] + 128 * sub
                    wo_ = 256 * PL[wb]
                    for j in range(4):
                        nc.tensor.matmul(
                            v_ps[:, 0:R],
                            xt[:, 2 * j:2 * j + 2, xo:xo + 128],
                            wt[:, 2 * j:2 * j + 2, wo_:wo_ + 256],
                            start=(n == 0), stop=(n == 11), perf_mode=DR)
                        n += 1
                va = v_all.rearrange("p (h n c) -> p h n c", h=HPC, n=NKC)
                nc.vector.tensor_copy(
                    va[:, :, c:c + 1, 0:64],
                    v_ps[:, 0:R].rearrange("p (h n c) -> p h n c", h=HPC, n=1))
              return
            return go

        # ---- out-proj pieces: (qvb, sc, nb) -> partial rows to OUT ----
        o_sbs = {}

        def op_item(sc, nb, tail=False, k=0):
            def go():
                if nb == 0:
                    o_sbs[sc] = osb.tile([128, D], mybir.dt.bfloat16, name="o_sb")
                o_ps = s_tile()[:, 0:512] if (tail and k % 3 != 2) else fv_tile()
                # 256-col halves: halves the PE-blocking granule so a filler
                # caught ahead of a just-ready scores pair delays it less
                for h in range(2):
                    for cn in range(2):
                        nc.tensor.matmul(
                            o_ps[:, 256 * h:256 * (h + 1)],
                            ctxT_sb[cn][:, 128 * sc:128 * (sc + 1)],
                            wo_sb[cn][:, 512 * nb + 256 * h:512 * nb + 256 * (h + 1)],
                            start=(cn == 0), stop=(cn == 1))
                dst = o_sbs[sc][:, 512 * nb:512 * (nb + 1)]
                if tail and k % 2 == 0:
                    nc.scalar.copy(dst, o_ps[:])
                else:
                    nc.vector.tensor_copy(dst, o_ps[:])
                # SWDGE (gpsimd) for output: its drain guarantees completion
                # before program end; HWDGE out-DMAs raced the epilogue barrier
                nc.gpsimd.dma_start(
                    out_d[128 * sc:128 * (sc + 1), 512 * nb:512 * (nb + 1)], dst)
                if nb == 1:
                    o_sbs.pop(sc)
            return go

        # ---- norm: drain cu fast (Pool den-broadcast + DVE copy), defer
        # recip+mul so the cu psum slot frees within ~1.4us of the last ctx
        # one DVE copy [0:65] frees the cu psum slot; Pool broadcasts the
        # den row (SBUF->SBUF; GPSIMD cannot read PSUM), DVE recip+mul after
        def norm_drain(c_ps, hh, last=False):
            def go():
              with tc.high_priority(offset=1000000):
                # den row sits at psum partition 64 (32-aligned); hop it into a
                # base-0 [1,512] tile since partition_broadcast only honors a
                # base-0 input on HW. The den hop and the v-rows copy read the
                # cu psum concurrently on different engines.
                den0 = normp.tile([1, 512], f32, name="den0")
                nc.vector.tensor_copy(den0[:], c_ps[hh][64:65, :])
                tmp = normp.tile([64, 512], f32, name="tmp")
                if last:
                    nc.scalar.copy(tmp[:], c_ps[hh][0:64, :])
                else:
                    nc.vector.tensor_copy(tmp[:], c_ps[hh][0:64, :])
                rin = normp.tile([64, 512], f32, name="rin")
                nc.gpsimd.partition_broadcast(rin[:], den0[0:1, :])
                return rin, tmp
            return go

        def norm_tail(rin, tmp, qvb, hp, hh, last=False):
            def go():
              with tc.high_priority(offset=500000):
                rb = normp.tile([64, 512], f32, name="rb")
                nc.vector.reciprocal_approx_fast(out=rb[:], in_=rin[:])
                dst = ctxT_sb[hp][64 * hh:64 * (hh + 1),
                                  512 * qvb:512 * (qvb + 1)]
                nc.vector.tensor_mul(dst, tmp[0:64, :], rb[:])
            return go

        # ---- injection schedule: slot g = global chunk index; items placed
        # by dependency deadline (scores of sweep s chunk c issue at slot
        # 16s+c-1), K-block h1 projections right after h0 to free x tiles ----
        NG = NSW * NKC
        inj = [[] for _ in range(NG + 1)]
        # sweep 0 is paced by the serialized input-DMA stream: place each
        # K/V-block projection at the slot where its transfer lands so a
        # stalled injection never delays already-feedable scores
        inj[0] += [qk_item("K2", "wk2", kT_sb, 0, 1, False),
                   v_item(0), v_item(1), v_item(2)]
        inj[1] += [qk_item("Q2", "wq2", qT_sb, 0, 1, True), v_item(3)]
        for blk in range(1, 4):
            inj[4 * blk - 2] += [qk_item("K2", "wk2", kT_sb, blk, 0, False)]
            inj[4 * blk - 1] += [qk_item("K2", "wk2", kT_sb, blk, 1, False)]
        for c in range(4, 16):
            inj[c - 1] += [v_item(c)]
        for blk in range(1, 4):
            inj[32 * blk - 12] += [qk_item("Q2", "wq2", qT_sb, blk, 0, True)]
            inj[32 * blk + 4] += [qk_item("Q2", "wq2", qT_sb, blk, 1, True)]
        # op pieces must be EMITTED after both norm tails (slots +34/+35):
        # Tile only syncs write->read in emission order, so an earlier-emitted
        # reader of ctxT races the deferred hh1 norm mul
        for qvb in range(3):
            for j in range(8):
                sc, nb = 4 * qvb + j // 2, j % 2
                inj[32 * qvb + 36 + 2 * j] += [op_item(sc, nb)]

        # ---- attention mainline ----
        def scores(g):
            s, c = divmod(g, NKC)
            qvb, hp = s // 2, s % 2
            s_ps = s_tile()
            for hh in range(2):
                nc.tensor.matmul(
                    s_ps[:, 512 * hh:512 * (hh + 1)],
                    kT_sb[hp][64 * hh:64 * (hh + 1), 128 * c:128 * (c + 1)],
                    qT_sb[hp][64 * hh:64 * (hh + 1), 512 * qvb:512 * (qvb + 1)],
                    start=True, stop=True, skip_group_check=True)
            return s_ps

        def ctx_mm(g, c_ps, expT):
            s, c = divmod(g, NKC)
            hp = (s // 2, s % 2)[1]
            # chunk 0 is one full-width start matmul (a start zeroes the whole
            # 2KB psum bank, so only ONE group may ever start in the tile);
            # later chunks accumulate in 256-col halves to shrink the granule
            # that can block a just-ready scores pair on the in-order PE
            for hh in range(2):
                gh = 2 * hp + hh
                if c == 0:
                    nc.tensor.matmul(
                        c_ps[hh][0:VW, :],
                        v_all[:, (gh * NKC + c) * VW:(gh * NKC + c + 1) * VW],
                        expT[:, 512 * hh:512 * (hh + 1)],
                        start=True, stop=False)
                else:
                    for h in range(2):
                        nc.tensor.matmul(
                            c_ps[hh][0:VW, 256 * h:256 * (h + 1)],
                            v_all[:, (gh * NKC + c) * VW:(gh * NKC + c + 1) * VW],
                            expT[:, 512 * hh + 256 * h:512 * hh + 256 * (h + 1)],
                            start=False, stop=(c == NKC - 1 and h == 1),
                            skip_group_check=True)

        # PE warmup: keep the tensor engine busy through the DMA prologue so
        # the pstate ramp completes before real work (cold PE runs 2-4x slow)
        scratch = sb.tile([128, 512], mybir.dt.bfloat16, name="scratch")
        nc.vector.memset(scratch[:], 0.0)

        def pe_dummy(n, tile_fn=None):
            for _ in range(n):
                d_ps = (tile_fn or fk_tile)()
                nc.tensor.matmul(
                    d_ps[:, 0:512], scratch[:, 0:128], scratch[:],
                    start=True, stop=True)

        pe_dummy(11)

        # prologue PE work (V moves to inj[0]: its DMA lands after Q0's)
        qk_item("K2", "wk2", kT_sb, 0, 0, False)()
        qk_item("Q2", "wq2", qT_sb, 0, 0, True)()

        with tc.high_priority(offset=1000000):
            sps_live = {0: scores(0)}
        cps_by_sweep = {}
        expT_live = {}
        for g in range(NG):
            s, c = divmod(g, NKC)
            qvb, hp = s // 2, s % 2
            if c == 0:
                cps_by_sweep[s] = [cu_tile() for _ in range(2)]
            with tc.high_priority(offset=1000000):
                expT = expp.tile([128, 1024], mybir.dt.bfloat16, name="expT")
                nc.scalar.activation(
                    expT[:], sps_live.pop(g)[:], Exp, scale=EXPSCALE)
            expT_live[g] = expT
            # scores(g+1) emitted before ctx(g-1): emission order is scheduler
            # priority, and ctx slotting between the two scores matmuls would
            # add ~200ns to every exp's critical path
            if g < NG - 1:
                with tc.high_priority(offset=1000000):
                    sps_live[g + 1] = scores(g + 1)
            if g > 0:
                ps, pc = divmod(g - 1, NKC)
                with tc.high_priority(offset=1000000):
                    ctx_mm(g - 1, cps_by_sweep[ps], expT_live.pop(g - 1)[:])
                if pc == NKC - 1:
                    c_ps = cps_by_sweep.pop(ps)
                    pqvb, php = ps // 2, ps % 2
                    for hh in range(2):
                        rin, tmp = norm_drain(c_ps, hh)()
                        inj[min(g + 2 + hh, NG)] += [
                            norm_tail(rin, tmp, pqvb, php, hh)]
            for it in inj[g]:
                it()

        # tail: last ctx + norm (Pool/DVE) with PE dummies covering the norm
        # latency so the out-proj runs at full clock, then out-proj qvb=3
        ctx_mm(NG - 1, cps_by_sweep[NSW - 1], expT_live.pop(NG - 1)[:])
        c_ps = cps_by_sweep.pop(NSW - 1)
        drains = [norm_drain(c_ps, hh, last=True)() for hh in range(2)]
        pe_dummy(15, tile_fn=s_tile)
        for hh in range(2):
            norm_tail(*drains[hh], 3, 1, hh, last=True)()
        for it in inj[NG]:
            it()
        o_big = sb.tile([128, 4, 1024], mybir.dt.bfloat16, name="o_big")
        for k, j in enumerate(range(8)):
            sc, nb = 12 + j // 2, j % 2
            o_ps = s_tile()[:, 0:512] if k % 3 != 2 else fv_tile()
            for cn in range(2):
                nc.tensor.matmul(
                    o_ps[:],
                    ctxT_sb[cn][:, 128 * sc:128 * (sc + 1)],
                    wo_sb[cn][:, 512 * nb:512 * (nb + 1)],
                    start=(cn == 0), stop=(cn == 1))
            dst = o_big[:, sc - 12, 512 * nb:512 * (nb + 1)]
            if k % 2 == 0:
                nc.scalar.copy(dst, o_ps[:])
            else:
                nc.vector.tensor_copy(dst, o_ps[:])
            if nb == 1:
                nc.gpsimd.dma_start(
                    out_d[128 * sc:128 * (sc + 1), :], o_big[:, sc - 12, :])

    nc.compile()
    return nc


def _hl(x):
    import ml_dtypes
    e4 = ml_dtypes.float8_e4m3
    hi = np.asarray(x, e4)
    lo = np.asarray(x - hi.astype(np.float32), e4)
    return hi, lo


def kernel(Q, K, V, wq, bq, wk, bk, wv, bv, wo, bo):
    from concourse.bass_utils import run_bass_kernel_spmd

    if "nc" not in _CACHE:
        _CACHE["nc"] = _build()
    nc = _CACHE["nc"]

    Q = np.asarray(Q, np.float32)
    K = np.asarray(K, np.float32)
    V = np.asarray(V, np.float32)
    xh = {}
    for nm, t in (("Q2", Q), ("K2", K), ("V2", V)):
        for b in range(B):
            hi, lo = _hl(t[b].T)
            packed = np.stack(
                [hi.reshape(D, 4, 512), lo.reshape(D, 4, 512)], axis=2)
            xh[(nm, b)] = np.ascontiguousarray(packed.reshape(D, 4096))
    wh = {}
    for nm, w in (("wq2", wq), ("wk2", wk), ("wv2", wv)):
        w = np.asarray(w, np.float32)
        for g in range(4):
            hi, lo = _hl(16.0 * w[g * R:(g + 1) * R].T)
            wh[(nm, g)] = np.ascontiguousarray(np.concatenate([hi, lo], axis=1))
    woT = [np.ascontiguousarray(np.asarray(wo, np.float32)[:, g * R:(g + 1) * R].T)
           for g in range(4)]
    bqs = [np.ascontiguousarray(
        16.0 * np.asarray(bq, np.float32)[g * R:(g + 1) * R, None])
        for g in range(4)]

    in_maps = []
    for c in range(NCORES):
        b, g = c // 4, c % 4
        m = {"woT": woT[g], "bq16": bqs[g]}
        for nm in ("Q2", "K2", "V2"):
            m[nm] = xh[(nm, b)]
        for nm in ("wq2", "wk2", "wv2"):
            m[nm] = wh[(nm, g)]
        in_maps.append(m)

    global _LAST_IN_MAPS
    _LAST_IN_MAPS = in_maps
    res = run_bass_kernel_spmd(nc, in_maps, core_ids=list(range(NCORES)))

    host_bias = (np.asarray(bv, np.float32) @ np.asarray(wo, np.float32).T
                 + np.asarray(bo, np.float32))
    out = np.zeros((B, S, D), np.float32)
    for c in range(NCORES):
        out[c // 4] += np.asarray(res.results[c]["OUT"], np.float32)
    out += host_bias[None, None, :]
    return out


# revision 56
# speedup vs baseline: 1.0117x; 1.0117x over previous
"""MultiHeadAttention TRN2 kernel: B=2, S=2048, D=1024, H=16, DK=64, 8 cores.

Sharding: core c handles batch b=c//4 and heads hg=(c%4)*4 .. +3 (data + head
parallel). Projections column-split by head; out-proj row-split; the
all-reduce after out-proj is done on host (sum of 4 partials per batch).

v2 design (cost-model driven):
- q/k/v projections run as fp8e4m3 hi+lo DoubleRow matmuls (2 contraction
  chunks per instr at 0.5 cyc/row): 3 product terms (xh*wh, xh*wl, xl*wh)
  give ~0.15% proj error while cutting proj PE time 25% and halving input
  DMA bytes. Weights are host-prescaled by 16 (fp8 subnormal avoidance);
  the 1/16 scales fold into the exp scale (q,k) and the ones column (v).
- Attention scores stay fp32r; exp output (probs) and v are bf16 (matmul
  still 1 cyc/row, ~0.3% extra error well inside the budget, and half the
  SBUF -- funding 13-deep expT and 8-deep input-block buffering). ctx is
  accumulated over chunks with a ones column (=16) appended to v for the
  softmax denominator.
- Streaming schedule: the first exp fires ~14us in (bq+wk+K0+wq+Q0 is
  the serialized-DMA critical prefix); projection/out-proj work is
  injected into per-chunk slots of the attention sweeps; scores are
  issued one chunk ahead of ctx so ACT rarely waits on PE.
- PSUM: s_ps0/1 (2 banks each, scores double-buffer), cu0/1 (ctx accum;
  fast norm drain via Pool den-broadcast frees them within ~1.5us), and
  fp0/1 (projections + out-proj, 2-deep so matmul overlaps evac) = 8
  banks. Emission order doubles as scheduler priority and as the psum
  tag-rotation order, so projection items are emitted in DMA-arrival
  order and the exp/scores/ctx mainline carries a large priority boost.
"""

from contextlib import ExitStack

import numpy as np

B, S, D, H, DK = 2, 2048, 1024, 16, 64
NCORES = 8
HPC = H // (NCORES // B)      # heads per core = 4
R = HPC * DK                  # local feats = 256
NKC = S // 128                # k-chunks per sweep = 16
NQB = 4                       # 512-wide q blocks
VW = 65                       # v chunk width (64 + ones col)
NSW = 8                       # sweeps = NQB * 2 head-pairs
EXPSCALE = 0.125 / 256.0      # qT,kT hold 16x values

_CACHE = {}
_LAST_IN_MAPS = None


def _build():
    import concourse.mybir as mybir
    import concourse.tile as tile
    from concourse import bacc

    f32 = mybir.dt.float32
    f32r = mybir.dt.float32r
    f8 = mybir.dt.float8e4
    DR = mybir.MatmulPerfMode.DoubleRow
    Exp = mybir.ActivationFunctionType.Exp
    Add = mybir.AluOpType.add

    nc = bacc.Bacc(
        "TRN2", target_bir_lowering=False, debug=False,
        enable_asserts=True, num_devices=NCORES,
    )

    # hi/lo fp8 planes interleaved at block granularity (3-dim DMA APs):
    # x2 [D, 4blk*(2plane*512)], w2 [D, 2plane*256] -- one DMA per block
    x_d = {}
    for nm in ("Q2", "K2", "V2"):
        x_d[nm] = nc.dram_tensor(nm, [D, 2 * S], f8, kind="ExternalInput").ap()
    w_d = {}
    for nm in ("wq2", "wk2", "wv2"):
        w_d[nm] = nc.dram_tensor(nm, [D, 2 * R], f8, kind="ExternalInput").ap()
    woT_d = nc.dram_tensor("woT", [R, D], f32r, kind="ExternalInput").ap()
    bq_d = nc.dram_tensor("bq16", [R, 1], f32, kind="ExternalInput").ap()
    out_d = nc.dram_tensor("OUT", [S, D], mybir.dt.bfloat16, kind="ExternalOutput").ap()

    with tile.TileContext(nc) as tc, ExitStack() as ctx:
        sb = ctx.enter_context(tc.tile_pool(name="sb", bufs=1))
        xin = ctx.enter_context(tc.tile_pool(name="xin", bufs=8))
        expp = ctx.enter_context(tc.tile_pool(name="expp", bufs=13))
        normp = ctx.enter_context(tc.tile_pool(name="normp", bufs=3))
        osb = ctx.enter_context(tc.tile_pool(name="osb", bufs=3))
        psum = ctx.enter_context(tc.tile_pool(name="psum", bufs=1, space="PSUM"))

        cnt = {"s": 0, "c": 0, "f": 0}

        def s_tile():
            i = cnt["s"]; cnt["s"] += 1
            return psum.tile([128, 1024], f32, name=f"s_ps{i % 2}", tag=f"s_ps{i % 2}")

        def fk_tile(w=512):
            i = cnt["f"]; cnt["f"] += 1
            return psum.tile([128, w], f32, name=f"fp{i % 2}", tag=f"fp{i % 2}")

        fv_tile = fk_tile

        def cu_tile():
            i = cnt["c"]; cnt["c"] += 1
            return psum.tile([128, 512], f32, name=f"cu{i % 2}", tag=f"cu{i % 2}")

        # ---- persistent SBUF ----
        w_sb = {nm: sb.tile([128, 8 * 2 * R], f8, name=f"{nm}_sb") for nm in w_d}
        bq_sb = sb.tile([128, 2], f32)
        wo_sb = [sb.tile([128, D], f32r, name=f"wo_sb{cn}") for cn in range(2)]
        qT_sb = [sb.tile([128, S], f32r, name=f"qT_sb{hp}") for hp in range(2)]
        kT_sb = [sb.tile([128, S], f32r, name=f"kT_sb{hp}") for hp in range(2)]
        v_all = sb.tile([128, HPC * NKC * VW], mybir.dt.bfloat16)
        ctxT_sb = [sb.tile([128, S], f32r, name=f"ctxT_sb{cn}") for cn in range(2)]

        onecol = sb.tile([128, 1], f32)
        nc.vector.memset(onecol[:], 16.0)   # absorbs the 1/16 of fp8 v scale
        vv = v_all.rearrange("p (n c) -> p n c", c=VW)[:, :, 64:65].rearrange(
            "p n c -> p (n c)")
        nc.vector.tensor_copy(vv, onecol[:].broadcast_to((128, HPC * NKC)))

        # ---- DMA: single queue, strict deadline order (the DMA engines
        # device serializes transfers globally at ~2.9us per merged block) ----
        def w_load(eng, nm):
            eng.dma_start(
                w_sb[nm].rearrange("p (d r2) -> p d r2", d=8),
                w_d[nm].rearrange("(d p) r2 -> p d r2", p=128))

        # x block tiles: [128, 8 dchunks, 2*512 (H cols | L cols)] fp8
        xtiles = {}

        def x_load(eng, nm, blk):
            t = xin.tile([128, 8, 1024], f8, name="xin", tag="xin")
            eng.dma_start(
                t[:],
                x_d[nm].rearrange("(d p) c -> p d c", p=128)[
                    :, :, 1024 * blk:1024 * (blk + 1)])
            xtiles[(nm, blk)] = t

        for hp in range(2):
            nc.sync.dma_start(bq_sb[:, hp:hp + 1], bq_d[128 * hp:128 * (hp + 1), :])
        w_load(nc.sync, "wk2")
        x_load(nc.sync, "K2", 0)
        w_load(nc.sync, "wq2")
        x_load(nc.sync, "Q2", 0)
        w_load(nc.sync, "wv2")
        x_load(nc.sync, "V2", 0)
        for blk in range(1, 4):
            x_load(nc.sync, "K2", blk)
            x_load(nc.sync, "V2", blk)
        x_load(nc.sync, "Q2", 1)
        for cn in range(2):
            nc.sync.dma_start(wo_sb[cn][:], woT_d[128 * cn:128 * (cn + 1), :])
        for blk in range(2, 4):
            x_load(nc.sync, "Q2", blk)

        # ---- fp8 hi/lo DoubleRow projection: 12 matmuls per psum tile ----
        TERMS = (("H", "H"), ("H", "L"), ("L", "H"))

        PL = {"H": 0, "L": 1}

        def proj_mms(p_ps, xnm, blk, wnm, hp, out_sl=slice(0, 512)):
            n = 0
            xt = xtiles[(xnm, blk)]
            wt = w_sb[wnm].rearrange("p (d r2) -> p d r2", d=8)
            for xa, wb in TERMS:
                wo_ = 256 * PL[wb] + 128 * hp
                xo = 512 * PL[xa]
                for j in range(4):
                    nc.tensor.matmul(
                        p_ps[:, out_sl],
                        wt[:, 2 * j:2 * j + 2, wo_:wo_ + 128],
                        xt[:, 2 * j:2 * j + 2, xo:xo + 512],
                        start=(n == 0), stop=(n == 11), perf_mode=DR)
                    n += 1

        def qk_item(xnm, wnm, dst_sb, blk, hp, bias):
            def go():
              from contextlib import nullcontext
              boost = (tc.high_priority(offset=500000) if xnm == "K2"
                       else nullcontext())
              with boost:
                p_ps = fk_tile()
                proj_mms(p_ps, xnm, blk, wnm, hp)
                dst = dst_sb[hp][:, 512 * blk:512 * (blk + 1)]
                # the evac gates the next sweep's first scores; boost it past
                # v-evacs (500k) but below the cu-critical norm drains (1M)
                with tc.high_priority(offset=750000):
                    if bias:
                        nc.vector.tensor_scalar(
                            dst, p_ps[:], bq_sb[:, hp:hp + 1], None, op0=Add)
                    else:
                        nc.vector.tensor_copy(dst, p_ps[:])
            return go

        def v_item(c):
            def go():
              with tc.high_priority(offset=500000):
                blk, sub = c // 4, c % 4
                v_ps = fv_tile(256)
                n = 0
                xt = xtiles[("V2", blk)]
                wt = w_sb["wv2"].rearrange("p (d r2) -> p d r2", d=8)
                for xa, wb in TERMS:
                    xo = 512 * PL[xa] + 128 * sub
                    wo_ = 256 * PL[wb]
                    for j in range(4):
                        nc.tensor.matmul(
                            v_ps[:, 0:R],
                            xt[:, 2 * j:2 * j + 2, xo:xo + 128],
                            wt[:, 2 * j:2 * j + 2, wo_:wo_ + 256],
                            start=(n == 0), stop=(n == 11), perf_mode=DR)
                        n += 1
                va = v_all.rearrange("p (h n c) -> p h n c", h=HPC, n=NKC)
                nc.vector.tensor_copy(
                    va[:, :, c:c + 1, 0:64],
                    v_ps[:, 0:R].rearrange("p (h n c) -> p h n c", h=HPC, n=1))
              return
            return go

        # ---- out-proj pieces: (qvb, sc, nb) -> partial rows to OUT ----
        o_sbs = {}

        def op_item(sc, nb, tail=False, k=0):
            def go():
                if nb == 0:
                    o_sbs[sc] = osb.tile([128, D], mybir.dt.bfloat16, name="o_sb")
                o_ps = s_tile()[:, 0:512] if (tail and k % 3 != 2) else fv_tile()
                # 256-col halves: halves the PE-blocking granule so a filler
                # caught ahead of a just-ready scores pair delays it less
                for h in range(2):
                    for cn in range(2):
                        nc.tensor.matmul(
                            o_ps[:, 256 * h:256 * (h + 1)],
                            ctxT_sb[cn][:, 128 * sc:128 * (sc + 1)],
                            wo_sb[cn][:, 512 * nb + 256 * h:512 * nb + 256 * (h + 1)],
                            start=(cn == 0), stop=(cn == 1))
                dst = o_sbs[sc][:, 512 * nb:512 * (nb + 1)]
                if tail and k % 2 == 0:
                    nc.scalar.copy(dst, o_ps[:])
                else:
                    nc.vector.tensor_copy(dst, o_ps[:])
                # SWDGE (gpsimd) for output: its drain guarantees completion
                # before program end; HWDGE out-DMAs raced the epilogue barrier
                nc.gpsimd.dma_start(
                    out_d[128 * sc:128 * (sc + 1), 512 * nb:512 * (nb + 1)], dst)
                if nb == 1:
                    o_sbs.pop(sc)
            return go

        # ---- norm: drain cu fast (Pool den-broadcast + DVE copy), defer
        # recip+mul so the cu psum slot frees within ~1.4us of the last ctx
        # one DVE copy [0:65] frees the cu psum slot; Pool broadcasts the
        # den row (SBUF->SBUF; GPSIMD cannot read PSUM), DVE recip+mul after
        def norm_drain(c_ps, hh, last=False):
            def go():
              with tc.high_priority(offset=1000000):
                # den row sits at psum partition 64 (32-aligned); hop it into a
                # base-0 [1,512] tile since partition_broadcast only honors a
                # base-0 input on HW. The den hop and the v-rows copy read the
                # cu psum concurrently on different engines.
                den0 = normp.tile([1, 512], f32, name="den0")
                nc.vector.tensor_copy(den0[:], c_ps[hh][64:65, :])
                tmp = normp.tile([64, 512], f32, name="tmp")
                if last:
                    nc.scalar.copy(tmp[:], c_ps[hh][0:64, :])
                else:
                    nc.vector.tensor_copy(tmp[:], c_ps[hh][0:64, :])
                rin = normp.tile([64, 512], f32, name="rin")
                nc.gpsimd.partition_broadcast(rin[:], den0[0:1, :])
                return rin, tmp
            return go

        def norm_tail(rin, tmp, qvb, hp, hh, last=False):
            def go():
              with tc.high_priority(offset=500000):
                rb = normp.tile([64, 512], f32, name="rb")
                nc.vector.reciprocal_approx_fast(out=rb[:], in_=rin[:])
                dst = ctxT_sb[hp][64 * hh:64 * (hh + 1),
                                  512 * qvb:512 * (qvb + 1)]
                nc.vector.tensor_mul(dst, tmp[0:64, :], rb[:])
            return go

        # ---- injection schedule: slot g = global chunk index; items placed
        # by dependency deadline (scores of sweep s chunk c issue at slot
        # 16s+c-1), K-block h1 projections right after h0 to free x tiles ----
        NG = NSW * NKC
        inj = [[] for _ in range(NG + 1)]
        # sweep 0 is paced by the serialized input-DMA stream: place each
        # K/V-block projection at the slot where its transfer lands so a
        # stalled injection never delays already-feedable scores
        inj[0] += [qk_item("K2", "wk2", kT_sb, 0, 1, False),
                   v_item(0), v_item(1), v_item(2)]
        inj[1] += [qk_item("Q2", "wq2", qT_sb, 0, 1, True), v_item(3)]
        for blk in range(1, 4):
            inj[4 * blk - 2] += [qk_item("K2", "wk2", kT_sb, blk, 0, False)]
            inj[4 * blk - 1] += [qk_item("K2", "wk2", kT_sb, blk, 1, False)]
        for c in range(4, 16):
            inj[c - 1] += [v_item(c)]
        for blk in range(1, 4):
            inj[32 * blk - 12] += [qk_item("Q2", "wq2", qT_sb, blk, 0, True)]
            inj[32 * blk + 4] += [qk_item("Q2", "wq2", qT_sb, blk, 1, True)]
        # op pieces must be EMITTED after both norm tails (slots +34/+35):
        # Tile only syncs write->read in emission order, so an earlier-emitted
        # reader of ctxT races the deferred hh1 norm mul
        for qvb in range(3):
            for j in range(8):
                sc, nb = 4 * qvb + j // 2, j % 2
                inj[32 * qvb + 36 + 2 * j] += [op_item(sc, nb)]

        # ---- attention mainline ----
        def scores(g):
            s, c = divmod(g, NKC)
            qvb, hp = s // 2, s % 2
            s_ps = s_tile()
            for hh in range(2):
                nc.tensor.matmul(
                    s_ps[:, 512 * hh:512 * (hh + 1)],
                    kT_sb[hp][64 * hh:64 * (hh + 1), 128 * c:128 * (c + 1)],
                    qT_sb[hp][64 * hh:64 * (hh + 1), 512 * qvb:512 * (qvb + 1)],
                    start=True, stop=True, skip_group_check=True)
            return s_ps

        def ctx_mm(g, c_ps, expT):
            s, c = divmod(g, NKC)
            hp = (s // 2, s % 2)[1]
            # chunk 0 is one full-width start matmul (a start zeroes the whole
            # 2KB psum bank, so only ONE group may ever start in the tile);
            # later chunks accumulate in 256-col halves to shrink the granule
            # that can block a just-ready scores pair on the in-order PE
            for hh in range(2):
                gh = 2 * hp + hh
                if c == 0:
                    nc.tensor.matmul(
                        c_ps[hh][0:VW, :],
                        v_all[:, (gh * NKC + c) * VW:(gh * NKC + c + 1) * VW],
                        expT[:, 512 * hh:512 * (hh + 1)],
                        start=True, stop=False)
                else:
                    for h in range(2):
                        nc.tensor.matmul(
                            c_ps[hh][0:VW, 256 * h:256 * (h + 1)],
                            v_all[:, (gh * NKC + c) * VW:(gh * NKC + c + 1) * VW],
                            expT[:, 512 * hh + 256 * h:512 * hh + 256 * (h + 1)],
                            start=False, stop=(c == NKC - 1 and h == 1),
                            skip_group_check=True)

        # PE warmup: keep the tensor engine busy through the DMA prologue so
        # the pstate ramp completes before real work (cold PE runs 2-4x slow)
        scratch = sb.tile([128, 512], mybir.dt.bfloat16, name="scratch")
        nc.vector.memset(scratch[:], 0.0)

        def pe_dummy(n, tile_fn=None):
            for _ in range(n):
                d_ps = (tile_fn or fk_tile)()
                nc.tensor.matmul(
                    d_ps[:, 0:512], scratch[:, 0:128], scratch[:],
                    start=True, stop=True)

        pe_dummy(11)

        # prologue PE work (V moves to inj[0]: its DMA lands after Q0's)
        qk_item("K2", "wk2", kT_sb, 0, 0, False)()
        qk_item("Q2", "wq2", qT_sb, 0, 0, True)()

        with tc.high_priority(offset=1000000):
            sps_live = {0: scores(0)}
        cps_by_sweep = {}
        expT_live = {}
        for g in range(NG):
            s, c = divmod(g, NKC)
            qvb, hp = s // 2, s % 2
            if c == 0:
                cps_by_sweep[s] = [cu_tile() for _ in range(2)]
            with tc.high_priority(offset=1000000):
                expT = expp.tile([128, 1024], mybir.dt.bfloat16, name="expT")
                nc.scalar.activation(
                    expT[:], sps_live.pop(g)[:], Exp, scale=EXPSCALE)
            expT_live[g] = expT
            # scores(g+1) emitted before ctx(g-1): emission order is scheduler
            # priority, and ctx slotting between the two scores matmuls would
            # add ~200ns to every exp's critical path
            if g < NG - 1:
                with tc.high_priority(offset=1000000):
                    sps_live[g + 1] = scores(g + 1)
            if g > 0:
                ps, pc = divmod(g - 1, NKC)
                with tc.high_priority(offset=1000000):
                    ctx_mm(g - 1, cps_by_sweep[ps], expT_live.pop(g - 1)[:])
                if pc == NKC - 1:
                    c_ps = cps_by_sweep.pop(ps)
                    pqvb, php = ps // 2, ps % 2
                    for hh in range(2):
                        rin, tmp = norm_drain(c_ps, hh)()
                        inj[min(g + 2 + hh, NG)] += [
                            norm_tail(rin, tmp, pqvb, php, hh)]
            for it in inj[g]:
                it()

        # tail: last ctx + norm (Pool/DVE) with PE dummies covering the norm
        # latency so the out-proj runs at full clock, then out-proj qvb=3
        ctx_mm(NG - 1, cps_by_sweep[NSW - 1], expT_live.pop(NG - 1)[:])
        c_ps = cps_by_sweep.pop(NSW - 1)
        drains = [norm_drain(c_ps, hh, last=True)() for hh in range(2)]
        pe_dummy(15, tile_fn=s_tile)
        for hh in range(2):
            norm_tail(*drains[hh], 3, 1, hh, last=True)()
        for it in inj[NG]:
            it()
        o_big = sb.tile([128, 4, 1024], mybir.dt.bfloat16, name="o_big")
        for k, j in enumerate(range(8)):
            sc, nb = 12 + j // 2, j % 2
            o_ps = s_tile()[:, 0:512] if k % 3 != 2 else fv_tile()
            for cn in range(2):
                nc.tensor.matmul(
                    o_ps[:],
                    ctxT_sb[cn][:, 128 * sc:128 * (sc + 1)],
                    wo_sb[cn][:, 512 * nb:512 * (nb + 1)],
                    start=(cn == 0), stop=(cn == 1))
            dst = o_big[:, sc - 12, 512 * nb:512 * (nb + 1)]
            if k % 2 == 0:
                nc.scalar.copy(dst, o_ps[:])
            else:
                nc.vector.tensor_copy(dst, o_ps[:])
            if nb == 1:
                nc.gpsimd.dma_start(
                    out_d[128 * sc:128 * (sc + 1), :], o_big[:, sc - 12, :])

    nc.compile()
    return nc


def _hl(x):
    import ml_dtypes
    e4 = ml_dtypes.float8_e4m3
    hi = np.asarray(x, e4)
    lo = np.asarray(x - hi.astype(np.float32), e4)
    return hi, lo


def kernel(Q, K, V, wq, bq, wk, bk, wv, bv, wo, bo):
    from concourse.bass_utils import run_bass_kernel_spmd

    if "nc" not in _CACHE:
        _CACHE["nc"] = _build()
    nc = _CACHE["nc"]

    Q = np.asarray(Q, np.float32)
    K = np.asarray(K, np.float32)
    V = np.asarray(V, np.float32)
    xh = {}
    for nm, t in (("Q2", Q), ("K2", K), ("V2", V)):
        for b in range(B):
            hi, lo = _hl(t[b].T)
            packed = np.stack(
                [hi.reshape(D, 4, 512), lo.reshape(D, 4, 512)], axis=2)
            xh[(nm, b)] = np.ascontiguousarray(packed.reshape(D, 4096))
    wh = {}
    for nm, w in (("wq2", wq), ("wk2", wk), ("wv2", wv)):
        w = np.asarray(w, np.float32)
        for g in range(4):
            hi, lo = _hl(16.0 * w[g * R:(g + 1) * R].T)
            wh[(nm, g)] = np.ascontiguousarray(np.concatenate([hi, lo], axis=1))
    woT = [np.ascontiguousarray(np.asarray(wo, np.float32)[:, g * R:(g + 1) * R].T)
           for g in range(4)]
    bqs = [np.ascontiguousarray(
        16.0 * np.asarray(bq, np.float32)[g * R:(g + 1) * R, None])
        for g in range(4)]

    in_maps = []
    for c in range(NCORES):
        b, g = c // 4, c % 4
        m = {"woT": woT[g], "bq16": bqs[g]}
        for nm in ("Q2", "K2", "V2"):
            m[nm] = xh[(nm, b)]
        for nm in ("wq2", "wk2", "wv2"):
            m[nm] = wh[(nm, g)]
        in_maps.append(m)

    global _LAST_IN_MAPS
    _LAST_IN_MAPS = in_maps
    res = run_bass_kernel_spmd(nc, in_maps, core_ids=list(range(NCORES)))

    host_bias = (np.asarray(bv, np.float32) @ np.asarray(wo, np.float32).T
                 + np.asarray(bo, np.float32))
    out = np.zeros((B, S, D), np.float32)
    for c in range(NCORES):
        out[c // 4] += np.asarray(res.results[c]["OUT"], np.float32)
    out += host_bias[None, None, :]
    return out
